# revision 1
# baseline (speedup 1.0000x reference)
"""Trainium2 Bass kernel for the LIDAR2D 4-direction selective-scan block.

Sharding: d_inner (E=512) split 8 ways (64 channels/core). The front
(matmuls + BottConv + projections) is cheap and duplicated on every core;
each core then runs the 4-direction selective scan for its 64 channels x
16 states and emits a partial out-projection (B, Dm, L). The host sums the
8 partials and transposes back to (B, L, Dm).

Scan layout per core: tiles of [128 partitions = (nsub in {0,1}) x (64
channels), free = L] — 8 tiles j=0..7 cover states n = 2j + nsub. The
recurrence h_t = exp(A*delta_t)*h_{t-1} + delta_t*B_t*u_t runs in a single
DVE tensor_tensor_scan per tile. y_t = C_t . h_t is a PE matmul with a
0/1 selection matrix contracting the two nsub rows per channel,
accumulated over j in PSUM. Direction permutations (reverse / spatial
transpose) are pure access-pattern tricks on reads/writes.
"""

import os
import sys

for _p in ("/opt/trn_rl_repo", os.path.expanduser("~/.axon_site/_ro/trn_rl_repo")):
    if os.path.isdir(_p) and _p not in sys.path:
        sys.path.insert(0, _p)

import numpy as np
import ml_dtypes

import concourse.bass as bass
import concourse.bacc as bacc
import concourse.mybir as mybir
from concourse.tile import TileContext
from concourse.bass_utils import run_bass_kernel_spmd

F32 = mybir.dt.float32
BF16 = mybir.dt.bfloat16
AF = mybir.ActivationFunctionType
OP = mybir.AluOpType

# Problem shape (hardcoded per the harness contract).
B, L, DM, E, N, R, MID, H, W = 2, 2304, 256, 512, 16, 16, 32, 48, 48
NCORES = 8
ESH = E // NCORES          # 64 channels per core
NJ = N // 2                # 8 scan tiles per (b, k); rows = (nsub, e_loc)
HALF = L // 2              # 1152, for PSUM-sized y accumulation

TRACE = bool(os.environ.get("KERNEL_TRACE"))
DEBUG = bool(os.environ.get("KERNEL_DEBUG"))
_LAST_EXEC_NS = None


def _install_profile_shim():
    """Make run_bass_kernel_spmd(trace=True) work in this container:
    register the NTFF hook (antenv.axon_hooks is absent here) and stub
    the S3 artifact upload."""
    import types
    try:
        from antenv.axon_hooks import get_axon_ntff_profile_hook  # noqa: F401
    except ImportError:
        import antenv
        mod = types.ModuleType("antenv.axon_hooks")
        mod._HOOK = None
        mod.set_axon_ntff_profile_hook = lambda h: setattr(mod, "_HOOK", h)
        mod.get_axon_ntff_profile_hook = lambda: mod._HOOK
        sys.modules["antenv.axon_hooks"] = mod
        antenv.axon_hooks = mod
        try:
            from trn_agent_boot.trn_boot import _ntff_profile_via_ctypes
            hook = _ntff_profile_via_ctypes("/opt/axon/libaxon_pjrt.so")
            if hook is not None:
                mod._HOOK = hook
        except Exception as e:  # pragma: no cover
            print(f"profile shim: hook install failed: {e}")
    import concourse.bass_utils as bu
    bu.upload_artifacts = lambda tmpdir: f"file://{tmpdir}"


def _chunks(total, step):
    out = []
    c0 = 0
    while c0 < total:
        out.append((c0, min(step, total - c0)))
        c0 += step
    return out


MM_CHUNKS = _chunks(L, 512)          # matmul free-dim chunks over full L
MM_CHUNKS_HALF = _chunks(HALF, 512)  # chunks within a 1152 half


def build_program():
    nc = bacc.Bacc()

    # ---- DRAM parameters (same shapes on every core; values differ) ----
    xT_d = nc.declare_dram_parameter("xT", [B, DM, L], F32, isOutput=False)
    posT_d = nc.declare_dram_parameter("posT", [DM, L], F32, isOutput=False)
    w_in_d = nc.declare_dram_parameter("w_in", [DM, E + ESH], F32, isOutput=False)
    w_pw1_d = nc.declare_dram_parameter("w_pw1", [E, MID], F32, isOutput=False)
    pw1b_d = nc.declare_dram_parameter("pw1b", [2 * MID, 1], F32, isOutput=False)
    dwtap_d = nc.declare_dram_parameter("dwtap", [2 * MID, 9], F32, isOutput=False)
    w_pw2_d = nc.declare_dram_parameter("w_pw2", [MID, E], F32, isOutput=False)
    w_xp_d = nc.declare_dram_parameter("w_xp", [E, R + 2 * N], F32, isOutput=False)
    w_dt_d = nc.declare_dram_parameter("w_dt", [R, ESH], F32, isOutput=False)
    spb_d = nc.declare_dram_parameter("spb", [ESH, 1], F32, isOutput=False)
    ascale_d = nc.declare_dram_parameter("ascale", [2 * ESH, NJ], F32, isOutput=False)
    dire_d = nc.declare_dram_parameter("dire", [ESH, 4], F32, isOutput=False)
    dp4_d = nc.declare_dram_parameter("dp4", [ESH, 1], F32, isOutput=False)
    dpb_d = nc.declare_dram_parameter("dpb", [ESH, 1], F32, isOutput=False)
    w_out_d = nc.declare_dram_parameter("w_out", [ESH, DM], F32, isOutput=False)
    sel_d = nc.declare_dram_parameter("sel", [2 * ESH, ESH], BF16, isOutput=False)
    out_d = nc.declare_dram_parameter("out", [B, DM, L], F32, isOutput=True)
    dbg = {}
    if DEBUG:
        dbg["xc"] = nc.declare_dram_parameter("dbg_xc", [B, ESH, L], F32,
                                              isOutput=True)
        dbg["delta"] = nc.declare_dram_parameter("dbg_delta", [B, ESH, L], F32,
                                                 isOutput=True)
        dbg["z"] = nc.declare_dram_parameter("dbg_z", [B, ESH, L], BF16,
                                             isOutput=True)
        dbg["y"] = nc.declare_dram_parameter("dbg_y", [B, ESH, L], F32,
                                             isOutput=True)
        dbg["bc"] = nc.declare_dram_parameter("dbg_bc", [2, B, N, L], BF16,
                                              isOutput=True)
        dbg["h1"] = nc.declare_dram_parameter("dbg_h1", [2 * MID, L], F32,
                                              isOutput=True)
        dbg["h2"] = nc.declare_dram_parameter("dbg_h2", [2 * MID, L], F32,
                                              isOutput=True)

    with TileContext(nc) as tc:
        with tc.tile_pool(name="const", bufs=1) as cp:
            # ---- load weights/constants ----
            w_in_t = [cp.tile([128, E + ESH], F32, tag=f"w_in{t}", name=f"w_in{t}") for t in range(2)]
            for t in range(2):
                nc.sync.dma_start(out=w_in_t[t][:], in_=w_in_d[t * 128:(t + 1) * 128, :])
            w_pw1_t = [cp.tile([128, MID], F32, tag=f"w_pw1{t}", name=f"w_pw1{t}") for t in range(4)]
            for t in range(4):
                nc.sync.dma_start(out=w_pw1_t[t][:], in_=w_pw1_d[t * 128:(t + 1) * 128, :])
            pw1b_t = cp.tile([2 * MID, 1], F32, tag="pw1b")
            nc.sync.dma_start(out=pw1b_t[:], in_=pw1b_d[:])
            dwtap_t = cp.tile([2 * MID, 9], F32, tag="dwtap")
            nc.sync.dma_start(out=dwtap_t[:], in_=dwtap_d[:])
            w_pw2_t = cp.tile([MID, E], F32, tag="w_pw2")
            nc.sync.dma_start(out=w_pw2_t[:], in_=w_pw2_d[:])
            w_xp_t = [cp.tile([128, R + 2 * N], F32, tag=f"w_xp{t}", name=f"w_xp{t}") for t in range(4)]
            for t in range(4):
                nc.sync.dma_start(out=w_xp_t[t][:], in_=w_xp_d[t * 128:(t + 1) * 128, :])
            w_dt_t = cp.tile([R, ESH], F32, tag="w_dt")
            nc.sync.dma_start(out=w_dt_t[:], in_=w_dt_d[:])
            spb_t = cp.tile([ESH, 1], F32, tag="spb")
            nc.sync.dma_start(out=spb_t[:], in_=spb_d[:])
            ascale_t = cp.tile([2 * ESH, NJ], F32, tag="ascale")
            nc.sync.dma_start(out=ascale_t[:], in_=ascale_d[:])
            dire_t = cp.tile([ESH, 4], F32, tag="dire")
            nc.sync.dma_start(out=dire_t[:], in_=dire_d[:])
            dp4_t = cp.tile([ESH, 1], F32, tag="dp4")
            nc.sync.dma_start(out=dp4_t[:], in_=dp4_d[:])
            dpb_t = cp.tile([ESH, 1], F32, tag="dpb")
            nc.sync.dma_start(out=dpb_t[:], in_=dpb_d[:])
            w_out_t = cp.tile([ESH, DM], F32, tag="w_out")
            nc.sync.dma_start(out=w_out_t[:], in_=w_out_d[:])
            sel_t = cp.tile([2 * ESH, ESH], BF16, tag="sel")
            nc.sync.dma_start(out=sel_t[:], in_=sel_d[:])

            # ---- persistent per-b products of the front ----
            with tc.tile_pool(name="persist", bufs=1) as pp:
                xc_sl = [pp.tile([ESH, L], F32, tag=f"xc_sl{b}", name=f"xc_sl{b}") for b in range(B)]
                delta_rep = [pp.tile([128, L], F32, tag=f"drep{b}", name=f"drep{b}") for b in range(B)]
                z_sl = [pp.tile([ESH, L], BF16, tag=f"z{b}", name=f"z{b}") for b in range(B)]
                y_acc = [pp.tile([ESH, L], F32, tag=f"yacc{b}", name=f"yacc{b}") for b in range(B)]

                with tc.tile_pool(name="bounce", bufs=1, space="DRAM") as bp:
                    bsrc_t = [bp.tile([N, L], BF16, tag=f"bsrc{b}", name=f"bsrc{b}") for b in range(B)]
                    csrc_t = [bp.tile([N, L], BF16, tag=f"csrc{b}", name=f"csrc{b}") for b in range(B)]

                    _front(nc, tc, locals())
                    _scan_and_out(nc, tc, locals())

    nc.finalize()
    return nc


def _front(nc, tc, env):
    """Positional embed + in-proj + BottConv + projections; fills
    xc_sl / delta_rep / z_sl / bsrc / csrc for both b."""
    g = env
    xT_d, posT_d = g["xT_d"], g["posT_d"]
    w_in_t, w_pw1_t, pw1b_t = g["w_in_t"], g["w_pw1_t"], g["pw1b_t"]
    dwtap_t, w_pw2_t, w_xp_t = g["dwtap_t"], g["w_pw2_t"], g["w_xp_t"]
    w_dt_t, spb_t = g["w_dt_t"], g["spb_t"]
    xc_sl, delta_rep, z_sl = g["xc_sl"], g["delta_rep"], g["z_sl"]
    bsrc_t, csrc_t = g["bsrc_t"], g["csrc_t"]
    # Every core's 64 channels sit at xc rows [0:64] of tile 0: the host
    # permutes pw2_w / W_xproj rows per core so the slice AP is uniform.

    with tc.tile_pool(name="front", bufs=1) as fp, \
         tc.tile_pool(name="fpsum", bufs=2, space="PSUM") as fps:
        h12 = fp.tile([2 * MID, L], F32, tag="h12", name="h12")
        with tc.tile_pool(name="frontio", bufs=1) as iop:
            pos_t = [iop.tile([128, L], F32, tag=f"pos{t}", name=f"pos{t}")
                     for t in range(2)]
            for t in range(2):
                nc.sync.dma_start(out=pos_t[t][:],
                                  in_=posT_d[t * 128:(t + 1) * 128, :])
            for b in range(B):
                # x^T + pos^T
                xin = [iop.tile([128, L], F32, tag=f"xin{t}", name=f"xin{t}")
                       for t in range(2)]
                for t in range(2):
                    nc.sync.dma_start(out=xin[t][:],
                                      in_=xT_d[b, t * 128:(t + 1) * 128, :])
                    nc.vector.tensor_tensor(out=xin[t][:], in0=xin[t][:],
                                            in1=pos_t[t][:], op=OP.add)
                # per 512-chunk: xz matmuls -> xh chunk tiles -> pw1 -> h12
                for (c0, nf) in MM_CHUNKS:
                    xhc = [iop.tile([128, 512], F32, tag=f"xhc{m}", bufs=2,
                                    name=f"xhc{m}") for m in range(4)]
                    for m in range(4):
                        ps = fps.tile([128, 512], F32, tag="fps", bufs=4, name="ps_xz")
                        for kt in range(2):
                            nc.tensor.matmul(ps[:, :nf],
                                             lhsT=w_in_t[kt][:, m * 128:(m + 1) * 128],
                                             rhs=xin[kt][:, c0:c0 + nf],
                                             start=(kt == 0), stop=(kt == 1))
                        nc.scalar.activation(out=xhc[m][:, :nf], in_=ps[:, :nf],
                                             func=AF.Copy)
                    psz = fps.tile([ESH, 512], F32, tag="fps", bufs=4, name="ps_z")
                    for kt in range(2):
                        nc.tensor.matmul(psz[:, :nf],
                                         lhsT=w_in_t[kt][:, E:E + ESH],
                                         rhs=xin[kt][:, c0:c0 + nf],
                                         start=(kt == 0), stop=(kt == 1))
                    nc.scalar.activation(out=z_sl[b][:, c0:c0 + nf],
                                         in_=psz[:, :nf], func=AF.Copy)
                    ps1 = fps.tile([2 * MID, 512], F32, tag="fps", bufs=4, name="ps_pw1")
                    for kt in range(4):
                        nc.tensor.matmul(ps1[b * MID:(b + 1) * MID, :nf],
                                         lhsT=w_pw1_t[kt][:],
                                         rhs=xhc[kt][:, :nf],
                                         start=(kt == 0), stop=(kt == 3))
                    nc.scalar.activation(
                        out=h12[b * MID:(b + 1) * MID, c0:c0 + nf],
                        in_=ps1[b * MID:(b + 1) * MID, :nf],
                        func=AF.Identity, bias=pw1b_t[b * MID:(b + 1) * MID, :])

        # depthwise 3x3 (both b at once, packed on partitions)
        acc = fp.tile([2 * MID, L], F32, tag="dwacc", name="dwacc")
        acc3 = acc[:].rearrange("p (h w) -> p h w", w=W)
        h3 = h12[:].rearrange("p (h w) -> p h w", w=W)
        nc.vector.tensor_scalar(out=acc[:], in0=h12[:],
                                scalar1=dwtap_t[:, 4:5], scalar2=None,
                                op0=OP.mult)
        for ky in range(3):
            for kx in range(3):
                if ky == 1 and kx == 1:
                    continue
                dy, dx = ky - 1, kx - 1
                h0, h1 = max(0, -dy), H - max(0, dy)
                w0, w1 = max(0, -dx), W - max(0, dx)
                nc.vector.scalar_tensor_tensor(
                    out=acc3[:, h0:h1, w0:w1],
                    in0=h3[:, h0 + dy:h1 + dy, w0 + dx:w1 + dx],
                    scalar=dwtap_t[:, ky * 3 + kx:ky * 3 + kx + 1],
                    in1=acc3[:, h0:h1, w0:w1],
                    op0=OP.mult, op1=OP.add)

        # matmul requires lhsT/rhs at the same base partition: move b=1's
        # dw rows down to base 0.
        acc_b = [acc[0:MID, :], None]
        acc1 = fp.tile([MID, L], F32, tag="acc1", name="acc1")
        nc.sync.dma_start(out=acc1[:], in_=acc[MID:2 * MID, :])
        acc_b[1] = acc1[:]
        if g["dbg"]:
            nc.sync.dma_start(out=g["dbg"]["h1"][:], in_=h12[:])
            nc.sync.dma_start(out=g["dbg"]["h2"][:], in_=acc[:])
        for b in range(B):
            # pw2 + SiLU -> xc (full E, 4 tiles)
            xc = [fp.tile([128, L], F32, tag=f"xc{m}", name=f"xc{m}")
                  for m in range(4)]
            for (c0, nf) in MM_CHUNKS:
                for m in range(4):
                    ps2 = fps.tile([128, 512], F32, tag="fps", bufs=4, name="ps_pw2")
                    nc.tensor.matmul(ps2[:, :nf],
                                     lhsT=w_pw2_t[:, m * 128:(m + 1) * 128],
                                     rhs=acc_b[b][:, c0:c0 + nf],
                                     start=True, stop=True)
                    nc.scalar.activation(out=xc[m][:, c0:c0 + nf], in_=ps2[:, :nf],
                                         func=AF.Silu)
            # e-slice of xc for this core (cross-partition move -> DMA)
            nc.sync.dma_start(out=xc_sl[b][:], in_=xc[0][0:ESH, :])
            # x_dbl = xc @ W_xproj  -> [48, L]
            xdbl = fp.tile([R + 2 * N, L], F32, tag="xdbl", name="xdbl")
            for (c0, nf) in MM_CHUNKS:
                ps3 = fps.tile([R + 2 * N, 512], F32, tag="fps", bufs=4, name="ps_xdbl")
                for kt in range(4):
                    nc.tensor.matmul(ps3[:, :nf],
                                     lhsT=w_xp_t[kt][:],
                                     rhs=xc[kt][:, c0:c0 + nf],
                                     start=(kt == 0), stop=(kt == 3))
                nc.scalar.activation(out=xdbl[:, c0:c0 + nf], in_=ps3[:, :nf],
                                     func=AF.Copy)
            # B/C rows to DRAM (bf16 cast via SWDGE) for later broadcast
            nc.gpsimd.dma_start(out=bsrc_t[b][:], in_=xdbl[R:R + N, :])
            nc.gpsimd.dma_start(out=csrc_t[b][:], in_=xdbl[R + N:R + 2 * N, :])
            # delta = softplus(dtr @ W_dt + 2*b_dt) into delta_rep rows 0:64
            for (c0, nf) in MM_CHUNKS:
                ps4 = fps.tile([ESH, 512], F32, tag="fps", bufs=4, name="ps_dt")
                nc.tensor.matmul(ps4[:, :nf], lhsT=w_dt_t[:],
                                 rhs=xdbl[0:R, c0:c0 + nf],
                                 start=True, stop=True)
                # softplus(v) = ln(1 + exp(v)); Softplus has no ACT table
                # set, but Exp and Ln share one. |v| stays < ~10 here so
                # exp cannot overflow.
                nc.scalar.activation(out=delta_rep[b][0:ESH, c0:c0 + nf],
                                     in_=ps4[:, :nf],
                                     func=AF.Exp, bias=spb_t[:])
                nc.scalar.activation(out=delta_rep[b][0:ESH, c0:c0 + nf],
                                     in_=delta_rep[b][0:ESH, c0:c0 + nf],
                                     func=AF.Ln, bias=1.0)
            # duplicate rows [0:64] -> [64:128]
            nc.sync.dma_start(out=delta_rep[b][ESH:2 * ESH, :],
                              in_=delta_rep[b][0:ESH, :])
            if g["dbg"]:
                dbg = g["dbg"]
                nc.sync.dma_start(out=dbg["xc"][b], in_=xc_sl[b][:])
                nc.sync.dma_start(out=dbg["delta"][b],
                                  in_=delta_rep[b][0:ESH, :])
                nc.sync.dma_start(out=dbg["z"][b], in_=z_sl[b][:])
                nc.sync.dma_start(out=dbg["bc"][0, b], in_=bsrc_t[b][:])
                nc.sync.dma_start(out=dbg["bc"][1, b], in_=csrc_t[b][:])


def _scan_and_out(nc, tc, env):
    g = env
    xc_sl, delta_rep, z_sl, y_acc = g["xc_sl"], g["delta_rep"], g["z_sl"], g["y_acc"]
    bsrc_t, csrc_t = g["bsrc_t"], g["csrc_t"]
    ascale_t, dire_t = g["ascale_t"], g["dire_t"]
    dp4_t, dpb_t = g["dp4_t"], g["dpb_t"]
    sel_t, w_out_t, out_d = g["sel_t"], g["w_out_t"], g["out_d"]

    with tc.tile_pool(name="scan", bufs=1) as sp, \
         tc.tile_pool(name="work", bufs=3) as wp, \
         tc.tile_pool(name="ypsum", bufs=1, space="PSUM") as yps, \
         tc.tile_pool(name="opsum", bufs=2, space="PSUM") as ops:
        for b in range(B):
            # init y_acc with the D*u skip term: Dp*(4*xc + sum_k dir_k)
            nc.scalar.activation(out=y_acc[b][:], in_=xc_sl[b][:],
                                 func=AF.Identity,
                                 bias=dpb_t[:], scale=dp4_t[:])
            dA_t = [None] * NJ
            for k in range(4):
                # u_k = perm_k(xc) + dir_k ; du = delta * u_k (bf16)
                xc3 = xc_sl[b][:].rearrange("p (h w) -> p h w", w=W)
                if k == 0:
                    src = xc3
                elif k == 1:
                    src = xc3[:, ::-1, ::-1]
                elif k == 2:
                    src = xc_sl[b][:].rearrange("p (h w) -> p w h", w=W)
                else:
                    src = xc_sl[b][:].rearrange("p (h w) -> p w h", w=W)[:, ::-1, ::-1]
                u_tmp = wp.tile([ESH, L], BF16, tag="u_tmp", bufs=2, name="u_tmp")
                u3 = u_tmp[:].rearrange("p (a c) -> p a c", c=W)
                nc.scalar.activation(out=u3, in_=src, func=AF.Identity,
                                     bias=dire_t[:, k:k + 1])
                du_rep = wp.tile([128, L], BF16, tag="du_rep", bufs=2, name="du_rep")
                nc.vector.tensor_tensor(out=du_rep[0:ESH, :],
                                        in0=delta_rep[b][0:ESH, :],
                                        in1=u_tmp[:], op=OP.mult)
                nc.sync.dma_start(out=du_rep[ESH:2 * ESH, :],
                                  in_=du_rep[0:ESH, :])
                ypsum = [yps.tile([ESH, HALF], F32, tag=f"yps{h}", name=f"yps{h}")
                         for h in range(2)]
                for j in range(NJ):
                    if k == 0:
                        dA_t[j] = sp.tile([128, L], F32 if j == 0 else BF16,
                                          tag=f"dA{j}", name=f"dA{j}")
                        nc.scalar.activation(out=dA_t[j][:], in_=delta_rep[b][:],
                                             func=AF.Exp,
                                             scale=ascale_t[:, j:j + 1])
                    B_t = wp.tile([128, L], BF16, tag="B_t", bufs=2, name="B_t")
                    C_t = wp.tile([128, L], BF16, tag="C_t", bufs=2, name="C_t")
                    for ns in range(2):
                        nc.sync.dma_start(
                            out=B_t[ns * ESH:(ns + 1) * ESH, :],
                            in_=bsrc_t[b][2 * j + ns:2 * j + ns + 1, :]
                            .to_broadcast((ESH, L)))
                        nc.sync.dma_start(
                            out=C_t[ns * ESH:(ns + 1) * ESH, :],
                            in_=csrc_t[b][2 * j + ns:2 * j + ns + 1, :]
                            .to_broadcast((ESH, L)))
                    dbu = wp.tile([128, L], BF16, tag="workA", name="dbu")
                    nc.vector.tensor_tensor(out=dbu[:], in0=du_rep[:],
                                            in1=B_t[:], op=OP.mult)
                    h_t = wp.tile([128, L], BF16, tag="workH", bufs=2, name="h_t")
                    nc.vector.tensor_tensor_scan(out=h_t[:], data0=dA_t[j][:],
                                                 data1=dbu[:], initial=0.0,
                                                 op0=OP.mult, op1=OP.add)
                    hc = wp.tile([128, L], BF16, tag="workA", name="hc")
                    nc.vector.tensor_tensor(out=hc[:], in0=h_t[:],
                                            in1=C_t[:], op=OP.mult)
                    for hh in range(2):
                        for (c0, nf) in MM_CHUNKS_HALF:
                            nc.tensor.matmul(
                                ypsum[hh][:, c0:c0 + nf],
                                lhsT=sel_t[:],
                                rhs=hc[:, hh * HALF + c0:hh * HALF + c0 + nf],
                                start=(j == 0), stop=(j == NJ - 1))
                # accumulate un-permuted ys_k into y_acc
                for hh in range(2):
                    pv = ypsum[hh][:]
                    if k == 0:
                        dst = y_acc[b][:, hh * HALF:(hh + 1) * HALF]
                        srcv = pv
                    elif k == 1:
                        dst = y_acc[b][:, (1 - hh) * HALF:(2 - hh) * HALF]
                        srcv = pv[:, ::-1]
                    elif k == 2:
                        # ys[i], i=a*48+b_ -> l = b_*48+a ; half hh: a in [24hh,24hh+24)
                        dst = y_acc[b][:].rearrange("p (bb a) -> p bb a", a=W)[
                            :, :, 24 * hh:24 * hh + 24]
                        srcv = pv.rearrange("p (a bb) -> p bb a", bb=W)
                    else:
                        dst = y_acc[b][:].rearrange("p (bb a) -> p bb a", a=W)[
                            :, :, 24 * (1 - hh):24 * (1 - hh) + 24]
                        srcv = pv.rearrange("p (a bb) -> p bb a", bb=W)[:, ::-1, ::-1]
                    nc.vector.tensor_tensor(out=dst, in0=srcv, in1=dst, op=OP.add)
            if g["dbg"]:
                nc.sync.dma_start(out=g["dbg"]["y"][b], in_=y_acc[b][:])
            # y_fin = y_acc * silu(z); out_partial = W_out^T @ y_fin
            sz = wp.tile([ESH, L], BF16, tag="u_tmp", bufs=2, name="sz")
            nc.scalar.activation(out=sz[:], in_=z_sl[b][:], func=AF.Silu)
            yv = wp.tile([ESH, L], F32, tag="yv", bufs=1, name="yv")
            nc.vector.tensor_tensor(out=yv[:], in0=y_acc[b][:], in1=sz[:],
                                    op=OP.mult)
            for m in range(2):
                osb = wp.tile([128, L], F32, tag="osb", bufs=2, name="osb")
                for (c0, nf) in MM_CHUNKS:
                    po = ops.tile([128, 512], F32, tag="out", name="ps_out")
                    nc.tensor.matmul(po[:, :nf],
                                     lhsT=w_out_t[:, m * 128:(m + 1) * 128],
                                     rhs=yv[:, c0:c0 + nf],
                                     start=True, stop=True)
                    nc.scalar.activation(out=osb[:, c0:c0 + nf], in_=po[:, :nf],
                                         func=AF.Copy)
                nc.sync.dma_start(out=out_d[b, m * 128:(m + 1) * 128, :],
                                  in_=osb[:])


def _host_prep(inputs):
    x = np.asarray(inputs["x"], np.float32)
    W_pos = np.asarray(inputs["W_pos"], np.float32)
    b_pos = np.asarray(inputs["b_pos"], np.float32)
    W_in = np.asarray(inputs["W_in"], np.float32)
    pw1_w = np.asarray(inputs["pw1_w"], np.float32)
    pw1_b = np.asarray(inputs["pw1_b"], np.float32)
    dw_w = np.asarray(inputs["dw_w"], np.float32)
    pw2_w = np.asarray(inputs["pw2_w"], np.float32)
    W_xproj = np.asarray(inputs["W_xproj"], np.float32)
    W_dt = np.asarray(inputs["W_dt"], np.float32)
    b_dt = np.asarray(inputs["b_dt"], np.float32)
    A_log = np.asarray(inputs["A_log"], np.float32)
    Dp = np.asarray(inputs["Dp"], np.float32)
    dir_emb = np.asarray(inputs["dir_emb"], np.float32)
    W_out = np.asarray(inputs["W_out"], np.float32)

    gy, gx = np.meshgrid(np.arange(H, dtype=np.float32),
                         np.arange(W, dtype=np.float32), indexing="ij")
    coords = np.stack([gy, gx], -1) / (H - 1) * 2 - 1
    pos = (coords.reshape(L, 2) @ W_pos + b_pos).astype(np.float32)

    common = {
        "xT": np.ascontiguousarray(x.transpose(0, 2, 1)),
        "posT": np.ascontiguousarray(pos.T),
        "w_pw1": np.ascontiguousarray(pw1_w.reshape(MID, E).T),
        "pw1b": np.ascontiguousarray(
            np.concatenate([pw1_b, pw1_b]).reshape(2 * MID, 1)),
        "dwtap": np.ascontiguousarray(
            np.concatenate([dw_w.reshape(MID, 9)] * 2, axis=0)),
    }
    w_pw2_base = pw2_w.reshape(E, MID).T  # (MID, E)
    A = -np.exp(A_log)  # (E, N)

    sel = np.zeros((2 * ESH, ESH), np.float32)
    for p in range(2 * ESH):
        sel[p, p % ESH] = 1.0
    sel = sel.astype(ml_dtypes.bfloat16)

    in_maps = []
    for c in range(NCORES):
        e0 = c * ESH
        sl = slice(e0, e0 + ESH)
        A_sl = A[sl]  # (64, 16)
        ascale = np.empty((2 * ESH, NJ), np.float32)
        for p in range(2 * ESH):
            for j in range(NJ):
                ascale[p, j] = A_sl[p % ESH, 2 * j + p // ESH]
        m = dict(common)
        # channel permutation putting this core's slice at rows [0:64]
        perm = np.concatenate([np.arange(e0, e0 + ESH),
                               np.arange(0, e0),
                               np.arange(e0 + ESH, E)])
        m["w_pw2"] = np.ascontiguousarray(w_pw2_base[:, perm])
        m["w_xp"] = np.ascontiguousarray(W_xproj[perm, :])
        m["w_in"] = np.ascontiguousarray(
            np.concatenate([W_in[:, :E], W_in[:, E + e0:E + e0 + ESH]], axis=1))
        m["w_dt"] = np.ascontiguousarray(W_dt[:, sl])
        m["spb"] = np.ascontiguousarray((2.0 * b_dt[sl]).reshape(ESH, 1))
        m["ascale"] = ascale
        m["dire"] = np.ascontiguousarray(dir_emb[:, sl].T)
        m["dp4"] = np.ascontiguousarray((4.0 * Dp[sl]).reshape(ESH, 1))
        m["dpb"] = np.ascontiguousarray(
            (Dp[sl] * dir_emb[:, sl].sum(0)).reshape(ESH, 1))
        m["w_out"] = np.ascontiguousarray(W_out[sl, :])
        m["sel"] = sel
        in_maps.append(m)
    return in_maps


_PROGRAM = None
_LAST_RESULTS = None
_LAST_INSTS = None


def _get_program():
    global _PROGRAM
    if _PROGRAM is None:
        _PROGRAM = build_program()
    return _PROGRAM


def kernel(**inputs):
    global _LAST_EXEC_NS, _LAST_RESULTS
    assert int(inputs["H"]) == H and int(inputs["W"]) == W
    in_maps = _host_prep(inputs)
    if TRACE:
        _install_profile_shim()
    res = run_bass_kernel_spmd(_get_program(), in_maps,
                               list(range(NCORES)), trace=TRACE)
    _LAST_EXEC_NS = res.exec_time_ns
    _LAST_RESULTS = res.results
    global _LAST_INSTS
    _LAST_INSTS = res.instructions_and_trace
    out = np.zeros((B, DM, L), np.float32)
    for r in res.results:
        out += np.asarray(r["out"], np.float32)
    return np.ascontiguousarray(out.transpose(0, 2, 1))

